# revision 6
# baseline (speedup 1.0000x reference)
"""Trainium2 Bass kernel for nn_ActorNetSpikingConv (spiking conv actor net).

Network (per timestep t of T=50, per sample):
    conv1: Conv1d(1->32, k=5, s=2, pad 1)  29 -> 14
    conv2: Conv1d(32->32, k=3, s=2, pad 1) 14 -> 7
    fc1:   224 -> 256
    fc2:   256 -> 2
  each followed by LIF dynamics:
    u = 0.5*u + syn;  v = 0.75*v*(1-s) + u;  s = (v > 0.5)
  output = sum_t s_fc2 / T

Implementation strategy (per core, pure data-parallel over batch):
  * Activations kept feature-major: [neuron, batch] on SBUF. Convs are
    expressed as dense matmuls with sparsity-expanded weights (contraction
    over the full input vector), so no im2col gathers are ever needed and
    every layer's spike output is directly the next layer's matmul rhs.
  * We store the spike COMPLEMENT sbar = 1 - s (exact in bf16). Downstream
    weights are negated and the bias absorbs W@1, so syn = theta + (-W)@sbar.
    theta (split hi/lo bf16) is added inside PSUM via a K=2 matmul against a
    ones vector; PSUM then holds the exact synaptic input and the LIF update
    needs no bias term at all:
        u  = (u*0.5) + psum          [scalar_tensor_tensor]
        v  = (v*0.75) * sbar         [scalar_tensor_tensor]
        v  = v + u                   [tensor_tensor]
        sbar = (v <= 0.5)            [tensor_scalar is_le]
  * conv1 runs in fp32 (moving operand is x, not bf16-exact); deeper layers
    use weights split into bf16 hi+lo pairs (2 accumulating matmuls, error
    ~2^-17) since their moving operand (sbar) is bf16-exact.
  * x is loaded batch-major once, transposed per step on the TensorEngine.
"""

import numpy as np
import ml_dtypes

import concourse.bass as bass
import concourse.tile as tile
from concourse import bacc, mybir
from concourse.bass_utils import run_bass_kernel_spmd

BF16 = ml_dtypes.bfloat16
F32 = np.float32

B, T, L, NACT, H = 8192, 50, 29, 2, 256
L1P, L2P = 14, 7  # conv output lengths
N_CORES = 8
BC = B // N_CORES  # per-core batch

OP = mybir.AluOpType
DT = mybir.dt


# ---------------------------------------------------------------- host prep

def _expand_w1(w1):
    """[32,1,5] -> dense [448, 29] (rows c*14+p), x-index l = 2p-1+k."""
    W = np.zeros((32 * L1P, L), np.float64)
    for c in range(32):
        for p in range(L1P):
            for k in range(5):
                l = 2 * p - 1 + k
                if 0 <= l < L:
                    W[c * L1P + p, l] = w1[c, 0, k]
    return W.astype(F32)


def _expand_w2(w2):
    """[32,32,3] -> dense [224, 448] (rows c2*7+p2, cols c1*14+l1)."""
    W = np.zeros((32 * L2P, 32 * L1P), np.float64)
    for c2 in range(32):
        for p2 in range(L2P):
            for k in range(3):
                l1 = 2 * p2 - 1 + k
                if 0 <= l1 < L1P:
                    for c1 in range(32):
                        W[c2 * L2P + p2, c1 * L1P + l1] = w2[c2, c1, k]
    return W.astype(F32)


def _hi_lo(a):
    hi = a.astype(BF16)
    lo = (a.astype(F32) - hi.astype(F32)).astype(BF16)
    return hi, lo


def _theta_pair(th):
    """fp32 theta vector [n] -> [2, n] bf16 (hi; lo)."""
    hi, lo = _hi_lo(th.astype(F32))
    return np.stack([hi, lo], 0)


def prep_consts(conv1_w, conv1_b, conv2_w, conv2_b, fc1_w, fc1_b, fc2_w, fc2_b):
    W1 = _expand_w1(np.asarray(conv1_w, F32))        # [448, 29]
    W2 = _expand_w2(np.asarray(conv2_w, F32))        # [224, 448]
    fw1 = np.asarray(fc1_w, F32)                     # [256, 224]
    fw2 = np.asarray(fc2_w, F32)                     # [2, 256]
    th1 = np.repeat(np.asarray(conv1_b, F32), L1P)                 # [448]
    th2 = np.repeat(np.asarray(conv2_b, F32), L2P) + W2.sum(1)     # [224]
    thf1 = np.asarray(fc1_b, F32) + fw1.sum(1)                     # [256]
    thf2 = np.asarray(fc2_b, F32) + fw2.sum(1)                     # [2]

    w2t_hi, w2t_lo = _hi_lo(np.ascontiguousarray((-W2).T))         # [448, 224]
    fw1t_hi, fw1t_lo = _hi_lo(np.ascontiguousarray((-fw1).T))      # [224, 256]
    fw2t_hi, fw2t_lo = _hi_lo(np.ascontiguousarray((-fw2).T))      # [256, 2]

    return {
        "w1t": np.ascontiguousarray(W1.T),           # [29, 448] fp32
        "w2t_hi": w2t_hi, "w2t_lo": w2t_lo,
        "fw1t_hi": fw1t_hi, "fw1t_lo": fw1t_lo,
        "fw2t_hi": fw2t_hi, "fw2t_lo": fw2t_lo,
        "th1": _theta_pair(th1), "th2": _theta_pair(th2),
        "thf1": _theta_pair(thf1), "thf2": _theta_pair(thf2),
        "ident": np.eye(128, dtype=F32),
    }


# ------------------------------------------------------------- bass program

# layer table: (name, P rows/chunk, M chunks, K chunks of prev sbar, prev P)
# L1 handled specially (fp32 conv from x).
def build_nc(Bc=BC, nsteps=T, engines=None):
    """Build + compile the per-core bass program."""
    eng = {"op1": "vector", "op2": "vector", "op3": "vector", "op4": "vector",
           "acc": "vector", "evac": "scalar"}
    if engines:
        eng.update(engines)

    NW = min(512, Bc)          # psum free width
    NH = Bc // NW              # batch halves
    BCH = Bc // 128            # 128-row batch chunks

    nc = bacc.Bacc("TRN2", target_bir_lowering=False, debug=False,
                   enable_asserts=True)

    xd = nc.dram_tensor("x", [Bc, nsteps * L], DT.float32, kind="ExternalInput")
    w1td = nc.dram_tensor("w1t", [29, 448], DT.float32, kind="ExternalInput")
    w2d = {hl: nc.dram_tensor(f"w2t_{hl}", [448, 224], DT.bfloat16, kind="ExternalInput")
           for hl in ("hi", "lo")}
    fw1d = {hl: nc.dram_tensor(f"fw1t_{hl}", [224, 256], DT.bfloat16, kind="ExternalInput")
            for hl in ("hi", "lo")}
    fw2d = {hl: nc.dram_tensor(f"fw2t_{hl}", [256, 2], DT.bfloat16, kind="ExternalInput")
            for hl in ("hi", "lo")}
    thd = {nm: nc.dram_tensor(nm, [2, n], DT.bfloat16, kind="ExternalInput")
           for nm, n in (("th1", 448), ("th2", 224), ("thf1", 256), ("thf2", 2))}
    identd = nc.dram_tensor("ident", [128, 128], DT.float32, kind="ExternalInput")
    outd = nc.dram_tensor("out", [Bc, 2], DT.float32, kind="ExternalOutput")

    with tile.TileContext(nc) as tc:
        with (
            tc.tile_pool(name="pconst", bufs=1) as pconst,
            tc.tile_pool(name="pstate", bufs=1) as pstate,
            tc.tile_pool(name="px", bufs=1) as px,
            tc.tile_pool(name="pxt", bufs=3) as pxt,
            tc.tile_pool(name="pout", bufs=2) as pout,
            tc.tile_pool(name="ppsum", bufs=1, space="PSUM") as pp,
        ):
            # ---- constants to SBUF
            w1sb = pconst.tile([29, 448], DT.float32, name="w1sb")
            nc.sync.dma_start(w1sb[:], w1td[:])
            ident = pconst.tile([128, 128], DT.float32, name="ident_sb")
            nc.sync.dma_start(ident[:], identd[:])
            w2sb = {}
            for hl in ("hi", "lo"):
                for kk in range(4):
                    t_ = pconst.tile([112, 224], DT.bfloat16, name=f"w2sb_{hl}{kk}")
                    nc.sync.dma_start(t_[:], w2d[hl][kk * 112:(kk + 1) * 112, :])
                    w2sb[hl, kk] = t_
            fw1sb = {}
            for hl in ("hi", "lo"):
                for kk in range(2):
                    t_ = pconst.tile([112, 256], DT.bfloat16, name=f"fw1sb_{hl}{kk}")
                    nc.sync.dma_start(t_[:], fw1d[hl][kk * 112:(kk + 1) * 112, :])
                    fw1sb[hl, kk] = t_
            fw2sb = {}
            for hl in ("hi", "lo"):
                for kk in range(2):
                    t_ = pconst.tile([128, 2], DT.bfloat16, name=f"fw2sb_{hl}{kk}")
                    nc.sync.dma_start(t_[:], fw2d[hl][kk * 128:(kk + 1) * 128, :])
                    fw2sb[hl, kk] = t_
            thsb = {}
            for nm, n in (("th1", 448), ("th2", 224), ("thf1", 256), ("thf2", 2)):
                t_ = pconst.tile([2, n], DT.bfloat16, name=f"{nm}_sb")
                nc.sync.dma_start(t_[:], thd[nm][:])
                thsb[nm] = t_
            ones = pconst.tile([2, NW], DT.bfloat16, name="ones_sb")
            nc.gpsimd.memset(ones[:], 1.0)

            # ---- x to SBUF (batch-major)
            xsb = []
            for c in range(BCH):
                t_ = px.tile([128, nsteps * L], DT.float32, name=f"xsb{c}")
                nc.sync.dma_start(t_[:], xd[c * 128:(c + 1) * 128, :])
                xsb.append(t_)

            # ---- state
            def st(name, p, mc, dt_):
                t_ = pstate.tile([p, mc * Bc], dt_, name=name)
                return t_

            layers = {
                "L1": dict(P=112, MC=4, th="th1"),
                "L2": dict(P=112, MC=2, th="th2"),
                "F1": dict(P=128, MC=2, th="thf1"),
                "F2": dict(P=2, MC=1, th="thf2"),
            }
            S = {}
            for nm, cfgl in layers.items():
                p, mc = cfgl["P"], cfgl["MC"]
                S[nm] = dict(
                    u=st(f"u_{nm}", p, mc, DT.float32),
                    v=st(f"v_{nm}", p, mc, DT.float32),
                    sb=st(f"sb_{nm}", p, mc, DT.bfloat16),
                )
                nc.gpsimd.memset(S[nm]["u"][:], 0.0)
                nc.gpsimd.memset(S[nm]["v"][:], 0.0)
                nc.gpsimd.memset(S[nm]["sb"][:], 1.0)
            accb = pstate.tile([2, Bc], DT.float32, name="accb")
            nc.gpsimd.memset(accb[:], 0.0)

            E = {k: getattr(nc, v) for k, v in eng.items()}

            def lif_ops(nm):
                """ops 2-4 (merged across M chunks) for layer nm."""
                u, v, sb = S[nm]["u"], S[nm]["v"], S[nm]["sb"]
                E["op2"].scalar_tensor_tensor(v[:], v[:], 0.75, sb[:], OP.mult, OP.mult)
                E["op3"].tensor_tensor(v[:], v[:], u[:], OP.add)
                E["op4"].tensor_scalar(sb[:], v[:], 0.5, None, OP.is_le)

            def op1(nm, m, nh, ps):
                u = S[nm]["u"]
                p = layers[nm]["P"]
                sl = u[0:p, m * Bc + nh * NW: m * Bc + nh * NW + NW]
                E["op1"].scalar_tensor_tensor(sl, sl, 0.5, ps[0:p, 0:NW],
                                              OP.mult, OP.add)

            def theta_mm(nm, m, ps, start):
                p = layers[nm]["P"]
                th = thsb[layers[nm]["th"]]
                nc.tensor.matmul(ps[0:p, 0:NW], th[0:2, m * p:(m + 1) * p],
                                 ones[0:2, 0:NW], start=start, stop=False)

            # ---- main time loop (fully unrolled)
            for t in range(nsteps):
                # transpose x_t -> xT [32, Bc] (rows 0:29 valid), feature-major
                xT = pxt.tile([32, Bc], DT.float32, name="xT")
                for c in range(BCH):
                    pt = pp.tile([29, 128], DT.float32, name="pt", tag="psmall", bufs=2)
                    nc.tensor.transpose(pt[:], xsb[c][:, t * L:(t + 1) * L],
                                        ident[:])
                    E["evac"].copy(xT[0:29, c * 128:(c + 1) * 128], pt[:])

                # conv1 (fp32) + L1
                for nh in range(NH):
                    for m in range(4):
                        ps = pp.tile([112, NW], DT.float32, name="pl1", tag="pl1", bufs=2)
                        theta_mm("L1", m, ps, start=True)
                        nc.tensor.matmul(
                            ps[:], w1sb[0:29, m * 112:(m + 1) * 112],
                            xT[0:29, nh * NW: nh * NW + NW],
                            start=False, stop=True)
                        op1("L1", m, nh, ps)
                lif_ops("L1")

                # conv2 (bf16 hi/lo) + L2
                for nh in range(NH):
                    for m in range(2):
                        ps = pp.tile([112, NW], DT.float32, name="pl2", tag="pl2", bufs=2)
                        theta_mm("L2", m, ps, start=True)
                        n_mm = 8
                        i = 0
                        for kk in range(4):
                            for hl in ("hi", "lo"):
                                i += 1
                                nc.tensor.matmul(
                                    ps[:], w2sb[hl, kk][:, m * 112:(m + 1) * 112],
                                    S["L1"]["sb"][0:112, kk * Bc + nh * NW: kk * Bc + nh * NW + NW],
                                    start=False, stop=(i == n_mm))
                        op1("L2", m, nh, ps)
                lif_ops("L2")

                # fc1 + F1
                for nh in range(NH):
                    for m in range(2):
                        ps = pp.tile([128, NW], DT.float32, name="pf1", tag="pf1", bufs=2)
                        theta_mm("F1", m, ps, start=True)
                        i = 0
                        for kk in range(2):
                            for hl in ("hi", "lo"):
                                i += 1
                                nc.tensor.matmul(
                                    ps[:], fw1sb[hl, kk][:, m * 128:(m + 1) * 128],
                                    S["L2"]["sb"][0:112, kk * Bc + nh * NW: kk * Bc + nh * NW + NW],
                                    start=False, stop=(i == 4))
                        op1("F1", m, nh, ps)
                lif_ops("F1")

                # fc2 + F2
                for nh in range(NH):
                    ps = pp.tile([2, NW], DT.float32, name="pf2", tag="psmall", bufs=2)
                    theta_mm("F2", 0, ps, start=True)
                    i = 0
                    for kk in range(2):
                        for hl in ("hi", "lo"):
                            i += 1
                            nc.tensor.matmul(
                                ps[:], fw2sb[hl, kk][:, 0:2],
                                S["F1"]["sb"][0:128, kk * Bc + nh * NW: kk * Bc + nh * NW + NW],
                                start=False, stop=(i == 4))
                    op1("F2", 0, nh, ps)
                lif_ops("F2")
                E["acc"].tensor_tensor(accb[:], accb[:], S["F2"]["sb"][:], OP.add)

            # ---- finalize: out[b, a] = accb.T (sbar count; host does (T-x)/T)
            for c in range(BCH):
                pa = pp.tile([128, 2], DT.float32, name="pa", tag="psmall", bufs=2)
                nc.tensor.transpose(pa[:], accb[0:2, c * 128:(c + 1) * 128],
                                    ident[0:2, 0:2])
                osb = pout.tile([128, 2], DT.float32, name="osb")
                nc.vector.tensor_copy(osb[:], pa[:])
                nc.sync.dma_start(outd[c * 128:(c + 1) * 128, :], osb[:])

    nc.compile()
    return nc


# ------------------------------------------------------------------- driver

_CACHE = {}


def _get_nc():
    if "nc" not in _CACHE:
        _CACHE["nc"] = build_nc()
    return _CACHE["nc"]


def kernel(x, conv1_w, conv1_b, conv2_w, conv2_b, fc1_w, fc1_b, fc2_w, fc2_b,
           batch_size=None, **_ignored):
    consts = prep_consts(conv1_w, conv1_b, conv2_w, conv2_b,
                         fc1_w, fc1_b, fc2_w, fc2_b)
    x2 = np.ascontiguousarray(np.asarray(x, F32).reshape(B, T * L))
    nc = _get_nc()
    in_maps = []
    for c in range(N_CORES):
        m = dict(consts)
        m["x"] = np.ascontiguousarray(x2[c * BC:(c + 1) * BC])
        in_maps.append(m)
    res = run_bass_kernel_spmd(nc, in_maps, core_ids=list(range(N_CORES)))
    sbar_cnt = np.concatenate([res.results[c]["out"] for c in range(N_CORES)], 0)
    return ((T - sbar_cnt) / T).astype(F32)


# revision 43
# speedup vs baseline: 57.6961x; 57.6961x over previous
"""Trainium2 Bass kernel for nn_ActorNetSpikingConv (spiking conv actor net).

Network (per timestep t of T=50, per sample):
    conv1: Conv1d(1->32, k=5, s=2, pad 1)  29 -> 14
    conv2: Conv1d(32->32, k=3, s=2, pad 1) 14 -> 7
    fc1:   224 -> 256
    fc2:   256 -> 2
  each followed by LIF dynamics:
    u = 0.5*u + syn;  v = 0.75*v*(1-s) + u;  s = (v > 0.5)
  output = sum_t s_fc2 / T

Implementation strategy (per core, pure data-parallel over batch):
  * Activations kept feature-major: [neuron, batch] on SBUF. Convs are
    expressed as dense matmuls with sparsity-expanded weights (contraction
    over the full input vector), so no im2col gathers are ever needed and
    every layer's spike output is directly the next layer's matmul rhs.
  * We store the spike COMPLEMENT sbar = 1 - s (exact in bf16). Downstream
    weights are negated and the bias absorbs W@1: syn = theta + (-W)@sbar.
  * theta rides inside the matmuls: sbar tiles carry two extra rows pinned
    at 1.0 (they are memset to 1.0 at init and the spike-update only writes
    rows [0:P)), and K-chunk 0 of each weight matrix carries theta_hi /
    theta_lo in those rows. PSUM then holds the exact synaptic input and
    the LIF update needs no bias term:
        u  = (u*0.5) + psum          [scalar_tensor_tensor]
        v  = (v*0.75) * sbar         [scalar_tensor_tensor]
        v  = v + u                   [tensor_tensor]
        sbar = (v <= 0.5)            [tensor_scalar is_le]
  * conv1 runs in fp32 (moving operand is x, not bf16-exact) with theta as
    an extra fp32 weight row against a ones-row in the transposed-x tile;
    deeper layers use weights split into bf16 hi+lo pairs (2 accumulating
    matmuls, error ~2^-17) since their moving operand (sbar) is bf16-exact.
  * x is loaded batch-major once, transposed per step on the TensorEngine.
  * The batch is processed in independent 512-column halves pipelined
    against each other so PE and DVE overlap across layers/steps.
"""

import numpy as np
import ml_dtypes

import concourse.bass as bass
import concourse.tile as tile
from concourse import bacc, mybir
from concourse.bass_utils import run_bass_kernel_spmd

BF16 = ml_dtypes.bfloat16
F32 = np.float32

B, T, L, NACT, H = 8192, 50, 29, 2, 256
L1P, L2P = 14, 7  # conv output lengths
N_CORES = 8
BC = B // N_CORES  # per-core batch

OP = mybir.AluOpType
DT = mybir.dt


# ---------------------------------------------------------------- host prep

def _expand_w1(w1):
    """[32,1,5] -> dense [448, 29] (rows c*14+p), x-index l = 2p-1+k."""
    W = np.zeros((32 * L1P, L), np.float64)
    for c in range(32):
        for p in range(L1P):
            for k in range(5):
                l = 2 * p - 1 + k
                if 0 <= l < L:
                    W[c * L1P + p, l] = w1[c, 0, k]
    return W.astype(F32)


def _expand_w2(w2):
    """[32,32,3] -> dense [224, 448] (rows c2*7+p2, cols c1*14+l1)."""
    W = np.zeros((32 * L2P, 32 * L1P), np.float64)
    for c2 in range(32):
        for p2 in range(L2P):
            for k in range(3):
                l1 = 2 * p2 - 1 + k
                if 0 <= l1 < L1P:
                    for c1 in range(32):
                        W[c2 * L2P + p2, c1 * L1P + l1] = w2[c2, c1, k]
    return W.astype(F32)


def _hi_lo(a):
    hi = a.astype(BF16)
    lo = (a.astype(F32) - hi.astype(F32)).astype(BF16)
    return hi, lo


def _chunked_lhsT(wT, kchunk, theta):
    """Split lhsT [K, M] into ceil(K/kchunk) chunks padded to kchunk+2 rows,
    bf16 hi/lo; theta (hi, lo) rows ride in chunk 0 of the HI stack.
    Returns (hi_stack, lo_stack) each [(nk*(kchunk+2)), M] bf16."""
    K, M = wT.shape
    nk = (K + kchunk - 1) // kchunk
    hi, lo = _hi_lo(wT)
    th_hi, th_lo = _hi_lo(theta.astype(F32))
    his, los = [], []
    for kk in range(nk):
        h = np.zeros((kchunk + 2, M), BF16)
        l_ = np.zeros((kchunk + 2, M), BF16)
        rows = slice(kk * kchunk, min((kk + 1) * kchunk, K))
        n = rows.stop - rows.start
        h[:n] = hi[rows]
        l_[:n] = lo[rows]
        if kk == 0:
            h[kchunk] = th_hi
            h[kchunk + 1] = th_lo
        his.append(h)
        los.append(l_)
    return np.concatenate(his, 0), np.concatenate(los, 0)


def prep_consts(conv1_w, conv1_b, conv2_w, conv2_b, fc1_w, fc1_b, fc2_w, fc2_b):
    W1 = _expand_w1(np.asarray(conv1_w, F32))        # [448, 29]
    W2 = _expand_w2(np.asarray(conv2_w, F32))        # [224, 448]
    fw1 = np.asarray(fc1_w, F32)                     # [256, 224]
    fw2 = np.asarray(fc2_w, F32)                     # [2, 256]
    th1 = np.repeat(np.asarray(conv1_b, F32), L1P)                 # [448]
    th2 = np.repeat(np.asarray(conv2_b, F32), L2P) + W2.sum(1)     # [224]
    thf1 = np.asarray(fc1_b, F32) + fw1.sum(1)                     # [256]
    thf2 = np.asarray(fc2_b, F32) + fw2.sum(1)                     # [2]

    # conv1 via bf16 hi/lo split of BOTH x and W1 (x is not bf16-exact).
    # The transposed-x tile xT [96, Bc] holds: rows 0:29 xh, row 32 ones,
    # row 33 ones, rows 64:93 xl, all other rows zero (partition bases must
    # be 0/32/64/96). Two matmuls:
    #   mm_a: lhsT w1t_a [96, 448] = {W1hi at 0:29 AND 64:93, th1hi@32,
    #         th1lo@33} x rhs xT[0:96]  ->  xh*W1hi + xl*W1hi + th1
    #   mm_b: lhsT w1t_b [29, 448] = W1lo x rhs xT[0:29]  ->  xh*W1lo
    W1hi, W1lo = _hi_lo(W1.T)                   # [29, 448]
    th1_hi, th1_lo = _hi_lo(th1)
    w1t_a = np.zeros((96, 448), BF16)
    w1t_a[0:29] = W1hi
    w1t_a[64:93] = W1hi
    w1t_a[32] = th1_hi
    w1t_a[33] = th1_lo
    w1t_b = np.ascontiguousarray(W1lo)

    w2t_hi, w2t_lo = _chunked_lhsT(np.ascontiguousarray((-W2).T), 112, th2)
    fw1t_hi, fw1t_lo = _chunked_lhsT(np.ascontiguousarray((-fw1).T), 112, thf1)

    fw2t_hi, fw2t_lo = _hi_lo(np.ascontiguousarray((-fw2).T))      # [256, 2]
    thf2_hi, thf2_lo = _hi_lo(thf2)

    return {
        "w1t_a": w1t_a, "w1t_b": w1t_b,               # [96|29, 448] bf16
        "w2t_hi": w2t_hi, "w2t_lo": w2t_lo,           # [456, 224] bf16 (4x114)
        "fw1t_hi": fw1t_hi, "fw1t_lo": fw1t_lo,       # [228, 256] bf16 (2x114)
        "fw2t_hi": fw2t_hi, "fw2t_lo": fw2t_lo,       # [256, 2] bf16
        "thf2": np.stack([thf2_hi, thf2_lo], 0),      # [2, 2] bf16
        "ident": np.eye(128, dtype=F32),
    }


# ------------------------------------------------------------- bass program

def build_nc(Bc=BC, nsteps=T, engines=None, repeat=1, bf16_state=()):
    """Build + compile the per-core bass program."""
    # NOTE: plain tensor_scalar (op4) is an illegal opcode on Pool (walrus
    # engine check); scalar_tensor_tensor and tensor_tensor are fine there.
    eng = {"op1": "vector", "op2": "gpsimd", "op3": "vector",
           "op4": "vector", "acc": "gpsimd", "evac": "scalar"}
    if engines:
        eng.update(engines)

    NW = min(512, Bc)          # psum free width
    NH = Bc // NW              # batch halves
    BCH = Bc // 128            # 128-row batch chunks
    CPH = NW // 128            # 128-row chunks per half

    nc = bacc.Bacc("TRN2", target_bir_lowering=False, debug=False,
                   enable_asserts=True)

    xd = nc.dram_tensor("x", [Bc, nsteps * L], DT.float32, kind="ExternalInput")
    w1ad = nc.dram_tensor("w1t_a", [96, 448], DT.bfloat16, kind="ExternalInput")
    w1bd = nc.dram_tensor("w1t_b", [29, 448], DT.bfloat16, kind="ExternalInput")
    w2d = {hl: nc.dram_tensor(f"w2t_{hl}", [456, 224], DT.bfloat16, kind="ExternalInput")
           for hl in ("hi", "lo")}
    fw1d = {hl: nc.dram_tensor(f"fw1t_{hl}", [228, 256], DT.bfloat16, kind="ExternalInput")
            for hl in ("hi", "lo")}
    fw2d = {hl: nc.dram_tensor(f"fw2t_{hl}", [256, 2], DT.bfloat16, kind="ExternalInput")
            for hl in ("hi", "lo")}
    thf2d = nc.dram_tensor("thf2", [2, 2], DT.bfloat16, kind="ExternalInput")
    identd = nc.dram_tensor("ident", [128, 128], DT.float32, kind="ExternalInput")
    outd = nc.dram_tensor("out", [Bc, 2], DT.float32, kind="ExternalOutput")

    with tile.TileContext(nc) as tc:
        with (
            tc.tile_pool(name="pconst", bufs=1) as pconst,
            tc.tile_pool(name="pstate", bufs=1) as pstate,
            tc.tile_pool(name="px", bufs=1) as px,
            tc.tile_pool(name="pout", bufs=2) as pout,
            tc.tile_pool(name="ppsum", bufs=1, space="PSUM") as pp,
        ):
            # ---- constants to SBUF
            w1asb = pconst.tile([96, 448], DT.bfloat16, name="w1asb")
            nc.sync.dma_start(w1asb[:], w1ad[:])
            w1bsb = pconst.tile([29, 448], DT.bfloat16, name="w1bsb")
            nc.sync.dma_start(w1bsb[:], w1bd[:])
            ident = pconst.tile([128, 128], DT.float32, name="ident_sb")
            nc.sync.dma_start(ident[:], identd[:])
            identb = pconst.tile([128, 128], DT.bfloat16, name="identb_sb")
            nc.scalar.copy(identb[:], ident[:])
            w2sb = {}
            for hl in ("hi", "lo"):
                for kk in range(4):
                    t_ = pconst.tile([114, 224], DT.bfloat16, name=f"w2sb_{hl}{kk}")
                    nc.sync.dma_start(t_[:], w2d[hl][kk * 114:(kk + 1) * 114, :])
                    w2sb[hl, kk] = t_
            fw1sb = {}
            for hl in ("hi", "lo"):
                for kk in range(2):
                    t_ = pconst.tile([114, 256], DT.bfloat16, name=f"fw1sb_{hl}{kk}")
                    nc.sync.dma_start(t_[:], fw1d[hl][kk * 114:(kk + 1) * 114, :])
                    fw1sb[hl, kk] = t_
            fw2sb = {}
            for hl in ("hi", "lo"):
                for kk in range(2):
                    t_ = pconst.tile([128, 2], DT.bfloat16, name=f"fw2sb_{hl}{kk}")
                    nc.sync.dma_start(t_[:], fw2d[hl][kk * 128:(kk + 1) * 128, :])
                    fw2sb[hl, kk] = t_
            thf2sb = pconst.tile([2, 2], DT.bfloat16, name="thf2_sb")
            nc.sync.dma_start(thf2sb[:], thf2d[:])
            ones = pconst.tile([2, NW], DT.bfloat16, name="ones_sb")
            nc.gpsimd.memset(ones[:], 1.0)

            # ---- x to SBUF (batch-major), split into bf16 hi + lo
            xh, xl = [], []
            with tc.tile_pool(name="pxtmp", bufs=2) as pxtmp:
                for c in range(BCH):
                    tmp = pxtmp.tile([128, nsteps * L], DT.float32, name="xtmp")
                    nc.sync.dma_start(tmp[:], xd[c * 128:(c + 1) * 128, :])
                    h_ = px.tile([128, nsteps * L], DT.bfloat16, name=f"xh{c}")
                    l_ = px.tile([128, nsteps * L], DT.bfloat16, name=f"xl{c}")
                    nc.scalar.copy(h_[:], tmp[:])
                    nc.gpsimd.tensor_tensor(l_[:], tmp[:], h_[:], OP.subtract)
                    xh.append(h_)
                    xl.append(l_)

            # ---- state (P = live rows; sbar tiles have 2 extra ones-rows)
            layers = {
                "L1": dict(P=112, MC=4),
                "L2": dict(P=112, MC=2),
                "F1": dict(P=128, MC=2),
                "F2": dict(P=2, MC=1),
            }
            S = {}
            for nm, cfgl in layers.items():
                p, mc = cfgl["P"], cfgl["MC"]
                psb = p + 2 if nm in ("L1", "L2") else p
                sdt = DT.bfloat16 if nm in bf16_state else DT.float32
                S[nm] = dict(
                    u=pstate.tile([p, mc * Bc], sdt, name=f"u_{nm}"),
                    v=pstate.tile([p, mc * Bc], sdt, name=f"v_{nm}"),
                    sb=pstate.tile([psb, mc * Bc], DT.bfloat16, name=f"sb_{nm}"),
                )
                nc.gpsimd.memset(S[nm]["u"][:], 0.0)
                nc.gpsimd.memset(S[nm]["v"][:], 0.0)
                nc.gpsimd.memset(S[nm]["sb"][:], 1.0)  # ones-rows stay 1.0
            accb = pstate.tile([2, Bc], DT.float32, name="accb")
            nc.gpsimd.memset(accb[:], 0.0)

            # persistent rotating transposed-x tiles [96, Bc] bf16:
            # rows 0:29 xh, rows 32+33 pinned 1.0 (theta rows), rows 64:93 xl,
            # everything else pinned 0 — per-step evacs only touch the x rows.
            xTs = []
            for i in range(3):
                t_ = pstate.tile([96, Bc], DT.bfloat16, name=f"xT{i}")
                nc.gpsimd.memset(t_[:], 0.0)
                nc.gpsimd.memset(t_[32:34, :], 1.0)
                xTs.append(t_)

            def E(op, nm=None):
                return getattr(nc, eng.get(f"{op}.{nm}", eng[op]))

            def half(ap, p, mc, nh):
                """AP for the nh-th NW-wide half of every column-block."""
                r = ap[0:p, :].rearrange("p (m b) -> p m b", m=mc)
                return r[:, :, nh * NW:(nh + 1) * NW]

            def lif_ops(nm, nh):
                cfgl = layers[nm]
                p, mc = cfgl["P"], cfgl["MC"]
                u, v, sb = S[nm]["u"], S[nm]["v"], S[nm]["sb"]
                vh, uh, sh = half(v, p, mc, nh), half(u, p, mc, nh), half(sb, p, mc, nh)
                # v = v*sbar (reset; plain tensor_tensor — Pool cannot run
                # TensorScalarPtr), then v = (v*0.75) + u (decay folded here)
                E("op2", nm).tensor_tensor(vh, vh, sh, OP.mult)
                E("op3", nm).scalar_tensor_tensor(vh, vh, 0.75, uh, OP.mult, OP.add)
                E("op4", nm).tensor_scalar(sh, vh, 0.5, None, OP.is_le)

            def op1(nm, m, nh, ps):
                u = S[nm]["u"]
                p = layers[nm]["P"]
                sl = u[0:p, m * Bc + nh * NW: m * Bc + nh * NW + NW]
                E("op1", nm).scalar_tensor_tensor(sl, sl, 0.5, ps[0:p, 0:NW],
                                                  OP.mult, OP.add)

            # ---- main time loop (fully unrolled; repeat>1 is for timing only)
            for t in [ti for _ in range(repeat) for ti in range(nsteps)]:
                # transpose xh_t / xl_t -> xT rows 0:29 / 64:93; four 128-col
                # transposes land in one psum bank tile, evacuated in one copy
                xT = xTs[t % 3]
                for nh in range(NH):
                    for src, base in ((xh, 0), (xl, 64)):
                        pt = pp.tile([29, NW], DT.bfloat16, name="pt",
                                     tag="pt", bufs=2)
                        for cc in range(CPH):
                            c = nh * CPH + cc
                            nc.tensor.transpose(
                                pt[0:29, cc * 128:(cc + 1) * 128],
                                src[c][:, t * L:(t + 1) * L], identb[:])
                        E("evac").copy(xT[base:base + 29, nh * NW:(nh + 1) * NW],
                                       pt[:])

                for nh in range(NH):
                    nwsl = slice(nh * NW, nh * NW + NW)
                    # conv1 (fp32, theta row 29 x ones row) + L1
                    for m in range(4):
                        ps = pp.tile([112, NW], DT.float32, name="pl1", tag="pl1", bufs=2)
                        nc.tensor.matmul(ps[:], w1asb[0:96, m * 112:(m + 1) * 112],
                                         xT[0:96, nwsl], start=True, stop=False)
                        nc.tensor.matmul(ps[:], w1bsb[0:29, m * 112:(m + 1) * 112],
                                         xT[0:29, nwsl], start=False, stop=True)
                        op1("L1", m, nh, ps)
                    lif_ops("L1", nh)

                    # conv2 (bf16 hi/lo, theta rides chunk0-hi) + L2
                    for m in range(2):
                        ps = pp.tile([112, NW], DT.float32, name="pl2", tag="pl2", bufs=2)
                        i = 0
                        for kk in range(4):
                            for hl in ("hi", "lo"):
                                i += 1
                                nc.tensor.matmul(
                                    ps[:], w2sb[hl, kk][:, m * 112:(m + 1) * 112],
                                    S["L1"]["sb"][0:114, kk * Bc + nh * NW: kk * Bc + nh * NW + NW],
                                    start=(i == 1), stop=(i == 8))
                        op1("L2", m, nh, ps)
                    lif_ops("L2", nh)

                    # fc1 + F1
                    for m in range(2):
                        ps = pp.tile([128, NW], DT.float32, name="pf1", tag="pf1", bufs=1)
                        i = 0
                        for kk in range(2):
                            for hl in ("hi", "lo"):
                                i += 1
                                nc.tensor.matmul(
                                    ps[:], fw1sb[hl, kk][:, m * 128:(m + 1) * 128],
                                    S["L2"]["sb"][0:114, kk * Bc + nh * NW: kk * Bc + nh * NW + NW],
                                    start=(i == 1), stop=(i == 4))
                        op1("F1", m, nh, ps)
                    lif_ops("F1", nh)

                    # fc2 + F2 (theta via explicit K=2 matmul against ones)
                    ps = pp.tile([2, NW], DT.float32, name="pf2", tag="psmall", bufs=1)
                    nc.tensor.matmul(ps[:], thf2sb[0:2, 0:2], ones[0:2, 0:NW],
                                     start=True, stop=False)
                    i = 0
                    for kk in range(2):
                        for hl in ("hi", "lo"):
                            i += 1
                            nc.tensor.matmul(
                                ps[:], fw2sb[hl, kk][:, 0:2],
                                S["F1"]["sb"][0:128, kk * Bc + nh * NW: kk * Bc + nh * NW + NW],
                                start=False, stop=(i == 4))
                    op1("F2", 0, nh, ps)
                    lif_ops("F2", nh)
                    E("acc").tensor_tensor(accb[0:2, nwsl], accb[0:2, nwsl],
                                           S["F2"]["sb"][0:2, nwsl], OP.add)

            # ---- finalize: out[b, a] = accb.T (sbar count; host does (T-x)/T)
            for c in range(BCH):
                pa = pp.tile([128, 2], DT.float32, name="pa", tag="psmall", bufs=1)
                nc.tensor.transpose(pa[:], accb[0:2, c * 128:(c + 1) * 128],
                                    ident[0:2, 0:2])
                osb = pout.tile([128, 2], DT.float32, name="osb")
                nc.vector.tensor_copy(osb[:], pa[:])
                nc.sync.dma_start(outd[c * 128:(c + 1) * 128, :], osb[:])

    nc.compile()
    return nc


# ------------------------------------------------------------------- driver

_CACHE = {}


def _get_nc():
    if "nc" not in _CACHE:
        _CACHE["nc"] = build_nc()
    return _CACHE["nc"]


def kernel(x, conv1_w, conv1_b, conv2_w, conv2_b, fc1_w, fc1_b, fc2_w, fc2_b,
           batch_size=None, **_ignored):
    consts = prep_consts(conv1_w, conv1_b, conv2_w, conv2_b,
                         fc1_w, fc1_b, fc2_w, fc2_b)
    x2 = np.ascontiguousarray(np.asarray(x, F32).reshape(B, T * L))
    nc = _get_nc()
    in_maps = []
    for c in range(N_CORES):
        m = dict(consts)
        m["x"] = np.ascontiguousarray(x2[c * BC:(c + 1) * BC])
        in_maps.append(m)
    res = run_bass_kernel_spmd(nc, in_maps, core_ids=list(range(N_CORES)))
    sbar_cnt = np.concatenate([res.results[c]["out"] for c in range(N_CORES)], 0)
    return ((T - sbar_cnt) / T).astype(F32)
